# revision 1
# baseline (speedup 1.0000x reference)
"""FNO1d Trainium2 kernel: 8-core SPMD, batch-sharded FNO + column-sharded token projection.

Self-contained: hardcodes all shapes. Two launches:
  A) per-core batch slice (8 of 64): lift -> 4x(spectral layer) -> proj1 -> proj2 -> y [8,4096]
  B) host gathers/transposes y; per-core output-column slice of tok projection.

Math: rFFT/irFFT with 32 modes == small DFT matmuls (F [4096,64], G [64,4096]).
Spectral branch in bf16 (mode truncation filters quantization noise);
pointwise/proj/token matmuls in fp32r. ~3.5e-4 rel err vs reference.

Perf notes: fwd DFT col-packed 2x and mode-mix + inverse-DFT row-packed 2x via
tile_position (concurrent subarray streams); pw + inverse accumulate into one
PSUM bank so the gelu reads a single psum tile; xbar layout transpose split
into t-halves so the fwd DFT overlaps the tail of the transpose; mix weights
prefetched on the ACT HWDGE queue (xbar owns the SP queue; transposes must
stay on ONE queue — cross-queue xbar races the global crossbar mode); oc
shuffle is 8 full-partition PE transposes + strided DVE copies; each layer's
xbar calls are emitted inside the PREVIOUS layer's z-loop right after the
producing b-pair group, so the transpose streams during z compute and the
next layer's fwd DFT starts immediately.
"""
import numpy as np
import ml_dtypes

import concourse.bass as bass
import concourse.mybir as mybir
import concourse.tile as tile
from concourse import bacc
from concourse import bass_utils
from concourse.masks import make_identity

B, T, W, MODES, NL = 64, 4096, 64, 32, 4
OUT_T = 4096
NC = 8            # cores
BL = B // NC      # batch per core = 8
NK = BL // 2      # b-pairs = 4
NTO = T // 128    # 32 t-chunks of 128
NCH = T // 512    # 8 t-chunks of 512
USL = OUT_T // NC  # 512 output cols per core in launch B

f32 = mybir.dt.float32
f32r = mybir.dt.float32r
bf16 = mybir.dt.bfloat16

_CACHE = {}


def _gelu_func():
    return mybir.ActivationFunctionType.Gelu


def _copy_func():
    for name in ("Copy", "Identity"):
        if hasattr(mybir.ActivationFunctionType, name):
            return getattr(mybir.ActivationFunctionType, name)
    raise RuntimeError("no copy activation")


def _build_a(stage=99):
    nc = bacc.Bacc("TRN2", target_bir_lowering=False, debug=False)

    x_c = nc.dram_tensor("x_c", [128, NK * T], f32, kind="ExternalInput").ap()
    fcat = nc.dram_tensor("fcat", [T, 64], bf16, kind="ExternalInput").ap()
    gcat = nc.dram_tensor("gcat", [64, T], bf16, kind="ExternalInput").ap()
    wab = nc.dram_tensor("wab", [NL, 2, 128, 16 * 128], bf16, kind="ExternalInput").ap()
    pwbd = nc.dram_tensor("pwbd", [NL, 128, 128], f32r, kind="ExternalInput").ap()
    p1bd = nc.dram_tensor("p1bd", [128, 128], f32r, kind="ExternalInput").ap()
    p2bd = nc.dram_tensor("p2bd", [128, 2], f32r, kind="ExternalInput").ap()
    liftw = nc.dram_tensor("liftw", [128, 1], f32, kind="ExternalInput").ap()
    liftb = nc.dram_tensor("liftb", [128, 1], f32, kind="ExternalInput").ap()
    pwb = nc.dram_tensor("pwb", [NL, 128, 1], f32, kind="ExternalInput").ap()
    p1b = nc.dram_tensor("p1b", [128, 1], f32, kind="ExternalInput").ap()

    y_out = nc.dram_tensor("y_out", [BL, T], f32, kind="ExternalOutput").ap()

    nlayers = NL if stage >= 90 else 1

    with tile.TileContext(nc) as tc:
        with tc.tile_pool(name="big", bufs=1) as bigp, \
             tc.tile_pool(name="wts", bufs=1) as wtp, \
             tc.tile_pool(name="mixw", bufs=1) as mixp, \
             tc.tile_pool(name="xs", bufs=2) as xsp, \
             tc.tile_pool(name="small", bufs=3) as smp, \
             tc.tile_pool(name="h2c", bufs=4) as h2p, \
             tc.tile_pool(name="psz", bufs=3, space="PSUM") as psz, \
             tc.tile_pool(name="psxf", bufs=1, space="PSUM") as psxf, \
             tc.tile_pool(name="psmix", bufs=1, space="PSUM") as psmix, \
             tc.tile_pool(name="pssm", bufs=2, space="PSUM") as pssm:

            hB = bigp.tile([128, NK * T], f32r, tag="hB")
            h16 = bigp.tile([128, NK * T], bf16, tag="h16")
            hA = bigp.tile([128, NTO * 512], bf16, tag="hA")
            hA4 = hA.rearrange("p (to k f) -> p to k f", to=NTO, k=NK)

            f_sb = wtp.tile([128, NTO * 64], bf16, tag="f_sb")
            nc.sync.dma_start(f_sb.rearrange("p (to m) -> p to m", to=NTO),
                              fcat.rearrange("(to p) m -> p to m", p=128))
            g_sb = wtp.tile([128, T], bf16, tag="g_sb")
            nc.sync.dma_start(g_sb[0:64, :], gcat[:])
            nc.sync.dma_start(g_sb[64:128, :], gcat[:])
            pwbd_sb = wtp.tile([128, NL * 128], f32r, tag="pwbd_sb")
            nc.sync.dma_start(pwbd_sb.rearrange("p (l m) -> p l m", l=NL),
                              pwbd.rearrange("l p m -> p l m"))
            p1bd_sb = wtp.tile([128, 128], f32r, tag="p1bd_sb")
            nc.sync.dma_start(p1bd_sb[:], p1bd[:])
            p2bd_sb = wtp.tile([128, 2], f32r, tag="p2bd_sb")
            nc.sync.dma_start(p2bd_sb[:], p2bd[:])
            ident = wtp.tile([128, 128], bf16, tag="ident")
            make_identity(nc, ident)
            liftw_sb = wtp.tile([128, 1], f32, tag="liftw_sb")
            nc.sync.dma_start(liftw_sb[:], liftw[:])
            liftb_sb = wtp.tile([128, 1], f32, tag="liftb_sb")
            nc.sync.dma_start(liftb_sb[:], liftb[:])
            pwb_sb = wtp.tile([128, NL], f32, tag="pwb_sb")
            nc.sync.dma_start(pwb_sb.rearrange("p (l o) -> p l o", l=NL),
                              pwb.rearrange("l p o -> p l o"))
            p1b_sb = wtp.tile([128, 1], f32, tag="p1b_sb")
            nc.sync.dma_start(p1b_sb[:], p1b[:])

            GELU = _gelu_func()

            # ---- lift ----
            for k in range(NK):
                for half in range(2):
                    xs = xsp.tile([128, 2048], f32, tag="xs")
                    sl0 = k * T + half * 2048
                    nc.sync.dma_start(xs[:], x_c[:, sl0:sl0 + 2048])
                    dst = hB[:, sl0:sl0 + 2048]
                    nc.vector.tensor_scalar(dst, xs[:], liftw_sb[:], liftb_sb[:],
                                            mybir.AluOpType.mult,
                                            mybir.AluOpType.add)
            def emit_xbar_k(k):
                nc.sync.dma_start_transpose(hA4[:, 0:16, k, :],
                                            h16[:, k * T:k * T + 2048])
                nc.sync.dma_start_transpose(hA4[:, 16:32, k, :],
                                            h16[:, k * T + 2048:(k + 1) * T])

            for k in range(NK):
                nc.vector.tensor_copy(h16[:, k * T:(k + 1) * T],
                                      hB[:, k * T:(k + 1) * T])
                if stage >= 1:
                    emit_xbar_k(k)

            # ---- layers ----
            for l in range(nlayers):
                if stage >= 4:
                    wa = mixp.tile([128, 16 * 128], bf16, tag="wa")
                    nc.scalar.dma_start(wa[:], wab[l, 0])
                    wb = mixp.tile([128, 16 * 128], bf16, tag="wb")
                    nc.scalar.dma_start(wb[:], wab[l, 1])
                if stage >= 2:
                    pxf = psxf.tile([128, 512], f32, tag="pxf")
                    for to in range(NTO):
                        half = to % 2
                        nc.tensor.matmul(pxf[half * 64:(half + 1) * 64, :],
                                         f_sb[:, to * 64:(to + 1) * 64],
                                         hA[:, to * 512:(to + 1) * 512],
                                         start=(to < 2), stop=(to >= NTO - 2),
                                         tile_position=(0, half * 64))
                    sxh = smp.tile([64, 512], f32, tag="sxh")
                    nc.vector.tensor_copy(sxh[:], pxf[64:128, :])
                    sxf = smp.tile([64, 512], bf16, tag="sxf")
                    nc.vector.tensor_tensor(sxf[:], pxf[0:64, :], sxh[:],
                                            mybir.AluOpType.add)
                if stage >= 3:
                    xfT = smp.tile([128, 512], bf16, tag="xfT")
                    for b in range(BL):
                        ptt = pssm.tile([64, 64], bf16, tag="sm")
                        nc.tensor.transpose(ptt[:], sxf[:, b * 64:(b + 1) * 64],
                                            ident[0:64, 0:64])
                        nc.vector.tensor_copy(xfT[0:64, b * 64:(b + 1) * 64], ptt[:])
                        nc.vector.tensor_copy(xfT[64:128, b * 64:(b + 1) * 64], ptt[:])
                if stage >= 4:
                    pmx = psmix.tile([128, 256], f32, tag="pmx")
                    pmx2 = psmix.tile([128, 256], f32, tag="pmx2")
                    for j in range(MODES // 2):
                        m0, m1 = 2 * j, 2 * j + 1
                        jb = slice(j * 128, (j + 1) * 128)
                        nc.tensor.matmul(pmx[:, m0 * 8:(m0 + 1) * 8],
                                         wa[0:64, jb],
                                         xfT[0:64, m0::64], start=True, stop=False,
                                         tile_position=(0, 0))
                        nc.tensor.matmul(pmx2[:, m1 * 8:(m1 + 1) * 8],
                                         wa[64:128, jb],
                                         xfT[64:128, m1::64], start=True, stop=False,
                                         tile_position=(64, 0))
                        nc.tensor.matmul(pmx[:, m0 * 8:(m0 + 1) * 8],
                                         wb[0:64, jb],
                                         xfT[0:64, 32 + m0::64], start=False, stop=True,
                                         tile_position=(0, 0))
                        nc.tensor.matmul(pmx2[:, m1 * 8:(m1 + 1) * 8],
                                         wb[64:128, jb],
                                         xfT[64:128, 32 + m1::64], start=False, stop=True,
                                         tile_position=(64, 0))
                    smx = smp.tile([128, 256], bf16, tag="smx")
                    nc.vector.tensor_copy(smx[:], pmx[:])
                    for j in range(MODES // 2):
                        m1 = 2 * j + 1
                        nc.vector.tensor_copy(smx[:, m1 * 8:(m1 + 1) * 8],
                                              pmx2[:, m1 * 8:(m1 + 1) * 8])
                if stage >= 5:
                    poc = pssm.tile([32, 1024], bf16, tag="sm")
                    for b in range(BL):
                        nc.tensor.transpose(poc[:, b * 128:(b + 1) * 128],
                                            smx[:, b::8], ident[:, :])
                    soc = smp.tile([128, 512], bf16, tag="soc")
                    pocv = poc.rearrange("p (b ro) -> p b ro", b=BL)
                    socv = soc.rearrange("p (b o) -> p b o", b=BL)
                    nc.vector.tensor_copy(socv[0:32, :, :], pocv[:, :, 0:64])
                    nc.vector.tensor_copy(socv[32:64, :, :], pocv[:, :, 64:128])
                    nc.vector.tensor_copy(socv[64:96, :, :], pocv[:, :, 0:64])
                    nc.vector.tensor_copy(socv[96:128, :, :], pocv[:, :, 64:128])
                if stage >= 6:
                    for kp in range(NK // 2):
                        k0, k1 = 2 * kp, 2 * kp + 1
                        for c in range(NCH):
                            sl0 = slice(k0 * T + c * 512, k0 * T + (c + 1) * 512)
                            sl1 = slice(k1 * T + c * 512, k1 * T + (c + 1) * 512)
                            pz0 = psz.tile([128, 512], f32, tag="pz")
                            pz1 = psz.tile([128, 512], f32, tag="pz")
                            nc.tensor.matmul(pz0[:], pwbd_sb[:, l * 128:(l + 1) * 128],
                                             hB[:, sl0], start=True, stop=False)
                            nc.tensor.matmul(pz1[:], pwbd_sb[:, l * 128:(l + 1) * 128],
                                             hB[:, sl1], start=True, stop=False)
                            nc.tensor.matmul(pz0[:], soc[0:64, 2 * k0 * 64: 2 * k0 * 64 + 128],
                                             g_sb[0:64, c * 512:(c + 1) * 512],
                                             start=False, stop=True, tile_position=(0, 0))
                            nc.tensor.matmul(pz1[:], soc[64:128, 2 * k1 * 64: 2 * k1 * 64 + 128],
                                             g_sb[64:128, c * 512:(c + 1) * 512],
                                             start=False, stop=True, tile_position=(64, 0))
                            nc.scalar.activation(hB[:, sl0], pz0[:], GELU,
                                                 bias=pwb_sb[:, l:l + 1], scale=1.0)
                            nc.scalar.activation(hB[:, sl1], pz1[:], GELU,
                                                 bias=pwb_sb[:, l:l + 1], scale=1.0)
                            if l < NL - 1:
                                nc.vector.tensor_copy(h16[:, sl0], hB[:, sl0])
                                nc.vector.tensor_copy(h16[:, sl1], hB[:, sl1])
                        if l < nlayers - 1 and stage >= 1:
                            emit_xbar_k(k0)
                            emit_xbar_k(k1)

            if stage >= 90:
                for k in range(NK):
                    for c in range(NCH):
                        pz = psz.tile([128, 512], f32, tag="pz")
                        sl = slice(k * T + c * 512, k * T + (c + 1) * 512)
                        nc.tensor.matmul(pz[:], p1bd_sb[:], hB[:, sl],
                                         start=True, stop=True)
                        h2c = h2p.tile([128, 512], f32r, tag="h2c")
                        nc.scalar.activation(h2c[:], pz[:], GELU,
                                             bias=p1b_sb[:], scale=1.0)
                        py = pssm.tile([2, 512], f32, tag="sm")
                        nc.tensor.matmul(py[:], p2bd_sb[:], h2c[:],
                                         start=True, stop=True)
                        syc = smp.tile([2, 512], f32, tag="syc")
                        nc.vector.tensor_copy(syc[:], py[:])
                        nc.sync.dma_start(
                            y_out[2 * k:2 * k + 2, c * 512:(c + 1) * 512], syc[:])
            else:
                dbg = xsp.tile([8, 4096], f32, tag="xs")
                nc.vector.tensor_copy(dbg[:], hB[0:8, 0:4096])
                nc.sync.dma_start(y_out[:], dbg[:])

    nc.compile()
    return nc


def _build_b():
    nc = bacc.Bacc("TRN2", target_bir_lowering=False, debug=False)
    yT = nc.dram_tensor("yT", [T, B], f32r, kind="ExternalInput").ap()
    tokw_c = nc.dram_tensor("tokw_c", [T, USL], f32r, kind="ExternalInput").ap()
    o_c = nc.dram_tensor("o_c", [B, USL], f32, kind="ExternalOutput").ap()

    with tile.TileContext(nc) as tc:
        with tc.tile_pool(name="sb", bufs=1) as pool, \
             tc.tile_pool(name="wstream", bufs=8) as wsp, \
             tc.tile_pool(name="ps", bufs=1, space="PSUM") as psp:
            yT_sb = pool.tile([128, NTO * B], f32r, tag="yT_sb")
            nc.sync.dma_start(yT_sb.rearrange("p (to b) -> p to b", to=NTO),
                              yT.rearrange("(to p) b -> p to b", p=128))
            po = psp.tile([B, USL], f32, tag="po")
            qs = [nc.sync, nc.scalar]
            for g in range(8):
                tw = wsp.tile([128, 4, USL], f32r, tag="tw")
                src = tokw_c.rearrange("(to p) u -> p to u", p=128)
                qs[g % 2].dma_start(tw[:], src[:, 4 * g:4 * (g + 1), :])
                for j in range(4):
                    to = 4 * g + j
                    nc.tensor.matmul(po[:], yT_sb[:, to * B:(to + 1) * B], tw[:, j, :],
                                     start=(to == 0), stop=(to == NTO - 1))
            so = pool.tile([B, USL], f32, tag="so")
            nc.vector.tensor_copy(so[:], po[:])
            nc.sync.dma_start(o_c[:], so[:])

    nc.compile()
    return nc


def _host_consts(lift_w, lift_b, spec_wr, spec_wi, pw_w, pw_b,
                 proj1_w, proj1_b, tok_w, tok_b):
    t = np.arange(T, dtype=np.float64)[:, None]
    m = np.arange(MODES, dtype=np.float64)[None, :]
    ang = 2.0 * np.pi * t * m / T
    Fcat = np.concatenate([np.cos(ang), -np.sin(ang)], axis=1)  # [T, 64]
    cm = np.full(MODES, 2.0 / T); cm[0] = 1.0 / T
    Gcat = np.concatenate([cm[:, None] * np.cos(ang.T),
                           -cm[:, None] * np.sin(ang.T)], axis=0)  # [64, T]
    fcat16 = Fcat.astype(ml_dtypes.bfloat16)
    gcat16 = Gcat.astype(ml_dtypes.bfloat16)

    wab = np.zeros((NL, 2, 128, 16 * 128), dtype=ml_dtypes.bfloat16)
    for l in range(NL):
        for mm in range(MODES):
            wr = spec_wr[l][:, :, mm]  # [i, o]
            wi = spec_wi[l][:, :, mm]
            rh = slice(0, 64) if mm % 2 == 0 else slice(64, 128)
            j = mm // 2
            wab[l, 0, rh, j * 128:j * 128 + 64] = wr
            wab[l, 0, rh, j * 128 + 64:(j + 1) * 128] = wi
            wab[l, 1, rh, j * 128:j * 128 + 64] = -wi
            wab[l, 1, rh, j * 128 + 64:(j + 1) * 128] = wr

    def blockdiag(wT):  # wT [i, o] -> [128, 128]
        out = np.zeros((128, 128), np.float32)
        out[0:64, 0:64] = wT
        out[64:128, 64:128] = wT
        return out

    pwbd = np.stack([blockdiag(pw_w[l].T) for l in range(NL)])  # [NL,128,128]
    p1bd = blockdiag(proj1_w.T)
    p2bd = np.zeros((128, 2), np.float32)
    p2bd[0:64, 0] = proj2_w_global[0]
    p2bd[64:128, 1] = proj2_w_global[0]

    liftw_col = np.tile(lift_w[:, 0], 2).reshape(128, 1).astype(np.float32)
    liftb_col = np.tile(lift_b, 2).reshape(128, 1).astype(np.float32)
    pwb_cols = np.stack([np.tile(pw_b[l], 2).reshape(128, 1) for l in range(NL)])
    p1b_col = np.tile(proj1_b, 2).reshape(128, 1).astype(np.float32)
    return (fcat16, gcat16, wab, pwbd, p1bd, p2bd,
            liftw_col, liftb_col, pwb_cols.astype(np.float32), p1b_col)


proj2_w_global = None


def kernel(x, lift_w, lift_b, spec_wr, spec_wi, pw_w, pw_b,
           proj1_w, proj1_b, proj2_w, proj2_b, tok_w, tok_b):
    global proj2_w_global
    proj2_w_global = np.asarray(proj2_w, np.float32)

    x = np.asarray(x, np.float32)
    if "a" not in _CACHE:
        _CACHE["a"] = _build_a()
    if "b" not in _CACHE:
        _CACHE["b"] = _build_b()

    (fcat16, gcat16, wab, pwbd, p1bd, p2bd,
     liftw_col, liftb_col, pwb_cols, p1b_col) = _host_consts(
        np.asarray(lift_w, np.float32), np.asarray(lift_b, np.float32),
        np.asarray(spec_wr, np.float32), np.asarray(spec_wi, np.float32),
        np.asarray(pw_w, np.float32), np.asarray(pw_b, np.float32),
        np.asarray(proj1_w, np.float32), np.asarray(proj1_b, np.float32),
        np.asarray(tok_w, np.float32), np.asarray(tok_b, np.float32))

    in_maps_a = []
    for c in range(NC):
        xc = x[c * BL:(c + 1) * BL]  # [8, T]
        xrep = np.repeat(xc.reshape(NK, 2, 1, T), 64, axis=2).reshape(NK, 128, T)
        xrep = np.ascontiguousarray(xrep.transpose(1, 0, 2).reshape(128, NK * T))
        in_maps_a.append({
            "x_c": xrep,
            "fcat": fcat16, "gcat": gcat16, "wab": wab,
            "pwbd": pwbd, "p1bd": p1bd, "p2bd": p2bd,
            "liftw": liftw_col, "liftb": liftb_col,
            "pwb": pwb_cols, "p1b": p1b_col,
        })
    res_a = bass_utils.run_bass_kernel_spmd(_CACHE["a"], in_maps_a,
                                            core_ids=list(range(NC)))
    y = np.concatenate([res_a.results[c]["y_out"] for c in range(NC)], axis=0)
    y = y + np.float32(np.asarray(proj2_b, np.float32)[0])
    yT = np.ascontiguousarray(y.T.astype(np.float32))  # [T, B]

    tok_w = np.asarray(tok_w, np.float32)
    tok_b = np.asarray(tok_b, np.float32)
    in_maps_b = []
    for c in range(NC):
        in_maps_b.append({
            "yT": yT,
            "tokw_c": np.ascontiguousarray(tok_w[c * USL:(c + 1) * USL, :].T),
        })
    res_b = bass_utils.run_bass_kernel_spmd(_CACHE["b"], in_maps_b,
                                            core_ids=list(range(NC)))
    out = np.concatenate([res_b.results[c]["o_c"] for c in range(NC)], axis=1)
    out = out + tok_b[None, :]
    return out.astype(np.float32)

